# revision 28
# baseline (speedup 1.0000x reference)
"""AdaAtt attention kernel for 8 Trainium2 NeuronCores (v3).

Pure data-parallel: batch B=2048 sharded 256 rows/core; weights replicated.
Per core (R=A=1024, G=49):

    fr  = relu(fake_region @ Wf1.T + bf1)
    fre = fr @ Wf2.T + bf2
    hl  = tanh(h_out @ Wh1.T + bh1)
    he  = hl @ Wh2.T + bh2
    scores[g] = Wa . tanh(embed[g] + he)     embed = [conv_feat_embed, fre]
    PI = softmax(scores)
    visAtt = sum_g PI[g] * img[g]            img = [fr, conv_feat]
    out = tanh((visAtt + hl) @ W2h.T + b2h)

v3 vs v2:
  - scores land directly in [b-part, g] PSUM via per-column fp8 DR matmuls
    (stationary = th chunk, moving = 8|wa| column); no DRAM bounce; exp runs
    on [128, 4..50] column tiles.
  - the big cfe+he adds are split DVE (c0-3) / Pool (c4-7); all non-tanh
    PSUM evacuations moved off ACT onto DVE tensor_scalar.
  - Wh2/Wf1/Wf2 + their denses in fp8 DoubleRow (x16 weights, x4
    activations prescale, folded into evac scalars); Wh1/W2h stay bf16.
  - nb-major schedule: half 0's softmax/visAtt/final dense overlap half
    1's tanh grid; vis/exp emission lags the grid by 3 blocks to avoid
    in-order engine stalls; chunked wh1 load starts the PE at ~1.5us.
  - cf streamed as 3 big DMAs per half (128 descriptors of ~25KB each).

Scales: hl8 = 4*hl, xf8 = xf, w*8 = 16*W.T, fr8 = 16*fr (slot 49 of cv;
exp bias -ln(16) on score col 49 compensates; its 1/16 in the softmax
denominator is undone explicitly), he psum = 64x, fre psum = 256x, score
psum = 8x true (awa8 = 8|wa|), exp scale 0.125.
Score/img column layout: 0..48 = conv g, 49 = fre/fr.
"""
import numpy as np
import ml_dtypes
from contextlib import ExitStack

import concourse.bass as bass
import concourse.mybir as mybir
import concourse.tile as tile
from concourse import bacc
from concourse.bass_utils import run_bass_kernel_spmd
from concourse.masks import make_identity

BF16 = mybir.dt.bfloat16
F32 = mybir.dt.float32
FP8 = mybir.dt.float8e4
AF = mybir.ActivationFunctionType
ALU = mybir.AluOpType
DR = mybir.MatmulPerfMode.DoubleRow

N_CORES = 8
B, R, A, G = 2048, 1024, 1024, 49
BS = B // N_CORES          # 256 rows per core
KC = R // 128              # 8 feature chunks
GB = 4                     # g's per block
NSC = G + 1                # score columns (49 conv + fre)
LN16 = float(np.log(16.0))
VIS_LAG = 3                # blocks of lag for exp+vis emission

# 13 blocks per nb-half: 12x4 conv + [48, fre]
P2_BLOCKS = [list(range(s, s + GB)) for s in range(0, 48, GB)] + [[48, -2]]
NBLK = len(P2_BLOCKS)
CV_RANGES = [(0, 24), (24, 48)]        # cvA: pairs 0-11, cvB: pairs 12-23

_CACHED_NC = None


def _build():
    nc = bacc.Bacc("TRN2", target_bir_lowering=False, debug=False,
                   num_devices=N_CORES)

    xh_d = nc.dram_tensor("xh", [R, BS], BF16, kind="ExternalInput").ap()
    xf8_d = nc.dram_tensor("xf8", [R, BS], FP8, kind="ExternalInput").ap()
    cfesb_d = nc.dram_tensor("cfesb", [A // 2, 2, G, 128], BF16,
                             kind="ExternalInput").ap()
    cfes8_d = nc.dram_tensor("cfes8", [A // 2, 2, G, 128], FP8,
                             kind="ExternalInput").ap()
    cf_d = nc.dram_tensor("cf", [BS, G, R], FP8, kind="ExternalInput").ap()
    wh1t_d = nc.dram_tensor("wh1t", [R, R], BF16, kind="ExternalInput").ap()
    w2ht_d = nc.dram_tensor("w2ht", [R, R], BF16, kind="ExternalInput").ap()
    wh2t8_d = nc.dram_tensor("wh2t8", [R, R], FP8, kind="ExternalInput").ap()
    wf1t8_d = nc.dram_tensor("wf1t8", [R, R], FP8, kind="ExternalInput").ap()
    wf2t8_d = nc.dram_tensor("wf2t8", [R, R], FP8, kind="ExternalInput").ap()
    # bias pack [128, 48]: bh1T | (sgn/64)T | (sgn*bh2)T | (16*bf1)T |
    #                      (sgn/256)T | (sgn*bf2)T
    bp_d = nc.dram_tensor("bp", [128, 48], F32, kind="ExternalInput").ap()
    awa_d = nc.dram_tensor("awa", [128, KC, 1], FP8, kind="ExternalInput").ap()
    b2h_d = nc.dram_tensor("b2h", [1, R], BF16, kind="ExternalInput").ap()
    out_d = nc.dram_tensor("out", [BS, R], F32, kind="ExternalOutput").ap()

    cfesb_r = cfesb_d.rearrange("(c p) nb g b -> p c nb g b", p=128)
    cfes8_r = cfes8_d.rearrange("(c p) nb g b -> p c nb g b", p=128)
    wh1_r = wh1t_d.rearrange("(c p) n -> p c n", p=128)
    w2h_r = w2ht_d.rearrange("(c p) n -> p c n", p=128)
    w8_r = {"wh2t8": wh2t8_d.rearrange("(c p) n -> p c n", p=128),
            "wf1t8": wf1t8_d.rearrange("(c p) n -> p c n", p=128),
            "wf2t8": wf2t8_d.rearrange("(c p) n -> p c n", p=128)}

    with ExitStack() as ctx:
        tc = ctx.enter_context(tile.TileContext(nc))
        consts = ctx.enter_context(tc.tile_pool(name="consts", bufs=1))
        wh1p = ctx.enter_context(tc.tile_pool(name="wh1p", bufs=KC))
        wpool = ctx.enter_context(tc.tile_pool(name="wpool", bufs=2))
        acts = ctx.enter_context(tc.tile_pool(name="acts", bufs=1))
        cfebp = ctx.enter_context(tc.tile_pool(name="cfebp", bufs=3))
        cfe8p = ctx.enter_context(tc.tile_pool(name="cfe8p", bufs=3))
        hsp = ctx.enter_context(tc.tile_pool(name="hsp", bufs=2))
        hsfp = ctx.enter_context(tc.tile_pool(name="hsfp", bufs=2))
        thp = ctx.enter_context(tc.tile_pool(name="thp", bufs=4))
        thfp = ctx.enter_context(tc.tile_pool(name="thfp", bufs=2))
        cvp = ctx.enter_context(tc.tile_pool(name="cvp", bufs=2))
        cvcp = ctx.enter_context(tc.tile_pool(name="cvcp", bufs=2))
        dgpp = ctx.enter_context(tc.tile_pool(name="dgpp", bufs=3))
        smallp = ctx.enter_context(tc.tile_pool(name="smallp", bufs=2))
        outp = ctx.enter_context(tc.tile_pool(name="outp", bufs=2))
        sumtp = ctx.enter_context(tc.tile_pool(name="sumtp", bufs=1))
        vpsp = ctx.enter_context(tc.tile_pool(name="vpsp", bufs=1))
        dense_ps = ctx.enter_context(tc.tile_pool(name="dense_ps", bufs=4,
                                                  space="PSUM"))
        sp_ps = ctx.enter_context(tc.tile_pool(name="sp_ps", bufs=2,
                                               space="PSUM"))
        vp_ps = ctx.enter_context(tc.tile_pool(name="vp_ps", bufs=1,
                                               space="PSUM"))

        # ---- constants ----
        ident = consts.tile([128, 128], BF16, name="ident")
        make_identity(nc, ident)
        ident8 = consts.tile([128, 128], FP8, name="ident8")
        make_identity(nc, ident8)
        ones1 = consts.tile([1, 128], BF16, name="ones1")
        nc.vector.memset(ones1, 1.0)
        nln16 = consts.tile([128, 1], F32, name="nln16")
        nc.vector.memset(nln16, -LN16)

        # ---- input loads (SP queue order = priority order) ----
        xh_sb = acts.tile([128, KC, BS], BF16, name="xh_sb")
        nc.sync.dma_start(out=xh_sb,
                          in_=xh_d.rearrange("(c p) b -> p c b", p=128))
        wh1c = []
        for kc in range(KC):
            w = wh1p.tile([128, 1, R], BF16, name=f"wh1c{kc}", tag="wh1c")
            nc.sync.dma_start(out=w, in_=wh1_r[:, kc:kc + 1, :])
            wh1c.append(w)
        bp = consts.tile([128, 48], F32, name="bp")
        nc.sync.dma_start(out=bp, in_=bp_d)
        awa = consts.tile([128, KC, 1], FP8, name="awa")
        nc.sync.dma_start(out=awa, in_=awa_d)
        b2h_sb = consts.tile([1, R], BF16, name="b2h_sb")
        nc.sync.dma_start(out=b2h_sb, in_=b2h_d)
        wh2t8 = wpool.tile([128, KC, R], FP8, name="wh2t8", tag="w")
        nc.sync.dma_start(out=wh2t8, in_=w8_r["wh2t8"])
        xf8_sb = acts.tile([128, KC, BS], FP8, name="xf8_sb")
        nc.sync.dma_start(out=xf8_sb,
                          in_=xf8_d.rearrange("(c p) b -> p c b", p=128))

        # ---- activations ----
        hl_T = acts.tile([128, KC, BS], BF16, name="hl_T")
        hl8 = acts.tile([128, KC, BS], FP8, name="hl8")       # 4*hl
        he_s = acts.tile([128, KC, BS], BF16, name="he_s")    # sgn*(he+bh2)
        fr8 = acts.tile([128, KC, BS], FP8, name="fr8")       # 16*fr
        fr_b = acts.tile([128, KC, BS], BF16, name="fr_b")    # 16*fr (bf16)
        fre_s = acts.tile([128, KC, BS], BF16, name="fre_s")  # sgn*(fre+bf2)
        expw = [acts.tile([128, 64], F32, name=f"expw{nb}") for nb in (0, 1)]

        # PE warm-up: the clock ramps 0.65->1.2->2.4GHz and resets on any
        # >100ns gap; filler matmuls on the identity hold it up through the
        # head's DMA-chasing gaps.
        warm = dense_ps.tile([128, 2, BS], F32, tag="dps", name="warm")

        def pe_warm(n):
            for _ in range(n):
                nc.tensor.matmul(warm[:, 0, 0:128], lhsT=ident, rhs=ident,
                                 start=True, stop=True, skip_group_check=True)

        pe_warm(10)
        # ---- hl dense: bf16; i-sequential groups (one start per bank at a
        # time — PSUM start marks the whole 2KB zero-region pending) ----
        for rp in range(KC // 2):
            ph = dense_ps.tile([128, 2, BS], F32, tag="dps", name=f"hlp{rp}")
            for i in range(2):
                rc = rp * 2 + i
                for kc in range(KC):
                    nc.tensor.matmul(
                        ph[:, i, :],
                        lhsT=wh1c[kc][:, 0, rc * 128:(rc + 1) * 128],
                        rhs=xh_sb[:, kc, :],
                        start=(kc == 0), stop=(kc == KC - 1))
                    if rp == 0 and i == 0:
                        pe_warm(8)   # fill the wh1-chunk arrival gaps
            for i in range(2):
                rc = rp * 2 + i
                nc.scalar.activation(
                    out=hl_T[:, rc, :], in_=ph[:, i, :],
                    func=AF.Tanh, bias=bp[:, rc:rc + 1])

        def dense8(w_sb, x8, evac):
            """fp8 DR dense: psum[rc] = sum_kp w[:,2kp:2kp+2,rc]*x8, evac'd."""
            for rp in range(KC // 2):
                ph = dense_ps.tile([128, 2, BS], F32, tag="dps", name="d8")
                for i in range(2):
                    rc = rp * 2 + i
                    for kp in range(KC // 2):
                        nc.tensor.matmul(
                            ph[:, i, :],
                            lhsT=w_sb[:, 2 * kp:2 * kp + 2,
                                      rc * 128:(rc + 1) * 128],
                            rhs=x8[:, 2 * kp:2 * kp + 2, :],
                            perf_mode=DR,
                            start=(kp == 0), stop=(kp == KC // 2 - 1))
                for i in range(2):
                    evac(rp * 2 + i, ph[:, i, :])

        # earliest cfes blocks: dispatch before the f-chain weight loads so
        # the grid can start as soon as he_s lands
        def load_cfe(nb, bi):
            blk = P2_BLOCKS[bi]
            ng = sum(1 for g in blk if g >= 0)
            cfeb = cfebp.tile([128, 4, GB, 128], BF16, tag="cfb", name="cfb")
            nc.sync.dma_start(out=cfeb[:, :, :ng, :],
                              in_=cfesb_r[:, :, nb, blk[0]:blk[0] + ng, :])
            cfe8 = cfe8p.tile([128, 4, GB, 128], FP8, tag="cf8", name="cf8")
            nc.sync.dma_start(out=cfe8[:, :, :ng, :],
                              in_=cfes8_r[:, :, nb, blk[0]:blk[0] + ng, :])
            return (cfeb, cfe8)

        cfes0 = {bi: load_cfe(0, bi) for bi in range(3)}

        # hl8 = 4*hl as fp8 (DVE)
        nc.vector.tensor_scalar_mul(hl8, hl_T, 4.0)
        pe_warm(30)   # hold the clock through the evac/convert gap
        # he dense (fp8): psum = 64*he ; he_s = (sgn/64)*psum + sgn*bh2
        dense8(wh2t8, hl8, lambda rc, ph: nc.vector.tensor_scalar(
            out=he_s[:, rc, :], in0=ph,
            scalar1=bp[:, 8 + rc:8 + rc + 1],
            scalar2=bp[:, 16 + rc:16 + rc + 1],
            op0=ALU.mult, op1=ALU.add))

        # f-chain is deferred into the early grid so the PE stays hot and
        # the head DMA window carries only wh1/wh2/xh/cfes
        def do_fchain():
            wf1t8 = wpool.tile([128, KC, R], FP8, name="wf1t8", tag="w")
            nc.sync.dma_start(out=wf1t8, in_=w8_r["wf1t8"])
            wf2t8 = wpool.tile([128, KC, R], FP8, name="wf2t8", tag="w")
            nc.sync.dma_start(out=wf2t8, in_=w8_r["wf2t8"])

            # fr dense (fp8): psum = 16*(xf@Wf1.T); fr8 = relu(psum + 16*bf1)
            def fr_evac(rc, ph):
                nc.vector.tensor_scalar(
                    out=fr8[:, rc, :], in0=ph,
                    scalar1=bp[:, 24 + rc:24 + rc + 1], scalar2=0.0,
                    op0=ALU.add, op1=ALU.max)
                nc.vector.tensor_scalar(
                    out=fr_b[:, rc, :], in0=ph,
                    scalar1=bp[:, 24 + rc:24 + rc + 1], scalar2=0.0,
                    op0=ALU.add, op1=ALU.max)
            dense8(wf1t8, xf8_sb, fr_evac)
            # fre dense (fp8): psum = 256*fre; fre_s = (sgn/256)*psum+sgn*bf2
            dense8(wf2t8, fr8, lambda rc, ph: nc.vector.tensor_scalar(
                out=fre_s[:, rc, :], in0=ph,
                scalar1=bp[:, 32 + rc:32 + rc + 1],
                scalar2=bp[:, 40 + rc:40 + rc + 1],
                op0=ALU.mult, op1=ALU.add))

        w2ht_box = {}

        # ---------------- per-half pipeline ----------------
        def load_cv_piece(nb, piece):
            bsl = slice(nb * 128, (nb + 1) * 128)
            if piece < 2:
                ga, gb_ = CV_RANGES[piece]
                cv = cvp.tile([128, gb_ - ga, R], FP8, tag="cv", name="cv")
                nc.gpsimd.dma_start(out=cv, in_=cf_d[bsl, ga:gb_, :])
                return cv
            cvc = cvcp.tile([128, 2, R], FP8, tag="cvc", name="cvc")
            nc.gpsimd.dma_start(out=cvc[:, 0:1, :], in_=cf_d[bsl, 48:49, :])
            return cvc

        def fill_fr(nb, cv_tiles):
            """cv slot 49 <- 64*fr via PE transpose + Pool evac."""
            bsl = slice(nb * 128, (nb + 1) * 128)
            cvc = cv_tiles[2]
            for rp in range(2):
                tp = dense_ps.tile([128, 4, 128], BF16, tag="dps", name="tpf")
                for i in range(4):
                    rc = rp * 4 + i
                    nc.tensor.transpose(tp[:, i, :], fr_b[:, rc, bsl], ident)
                nc.vector.tensor_scalar_mul(
                    cvc[:, 1, rp * 512:(rp + 1) * 512], tp, 1.0)

        def grid_block(nb, bi, cfe, spT):
            """adds + tanh + score columns for one block (no exp here)."""
            blk = P2_BLOCKS[bi]
            bsl = slice(nb * 128, (nb + 1) * 128)
            ng = sum(1 for g in blk if g >= 0)
            gb = len(blk)
            cfeb, cfe8 = cfe
            hs = hsp.tile([128, KC, GB, 128], BF16, tag="hs", name="hs")
            # c0-3: bf16 (DVE 2x); c4-7: fp8, split DVE/Pool by measured rates
            he_lo = he_s[:, 0:4, bsl].unsqueeze(2).to_broadcast(
                [128, 4, ng, 128])
            nc.vector.tensor_add(hs[:, 0:4, :ng, :], cfeb[:, :, :ng, :],
                                 he_lo)
            dc8 = 3 if bi % 2 == 0 else 2
            he_m = he_s[:, 4:4 + dc8, bsl].unsqueeze(2).to_broadcast(
                [128, dc8, ng, 128])
            nc.vector.tensor_add(hs[:, 4:4 + dc8, :ng, :],
                                 cfe8[:, 0:dc8, :ng, :], he_m)
            he_hi = he_s[:, 4 + dc8:8, bsl].unsqueeze(2).to_broadcast(
                [128, 4 - dc8, ng, 128])
            nc.gpsimd.tensor_add(hs[:, 4 + dc8:8, :ng, :],
                                 cfe8[:, dc8:4, :ng, :], he_hi)
            th = thp.tile([128, KC, GB, 128], FP8, tag="th", name="th")
            nc.scalar.activation(out=th[:, :, :ng, :], in_=hs[:, :, :ng, :],
                                 func=AF.Tanh)
            th_f = None
            if gb > ng:  # fre slot
                hsf = hsfp.tile([128, KC, 128], BF16, tag="hsf", name="hsf")
                nc.vector.tensor_add(hsf, fre_s[:, :, bsl], he_s[:, :, bsl])
                th_f = thfp.tile([128, KC, 128], FP8, tag="thf", name="thf")
                nc.scalar.activation(out=th_f, in_=hsf, func=AF.Tanh)
            # score columns: per g, 4 accumulating fp8 DR matmuls
            for gl in range(gb):
                col = NSC - 1 if blk[gl] == -2 else blk[gl]
                for kp in range(KC // 2):
                    lhsT = (th[:, 2 * kp:2 * kp + 2, gl, :] if gl < ng
                            else th_f[:, 2 * kp:2 * kp + 2, :])
                    nc.tensor.matmul(
                        spT[:, col:col + 1], lhsT=lhsT,
                        rhs=awa[:, 2 * kp:2 * kp + 2, :],
                        perf_mode=DR,
                        start=(kp == 0), stop=(kp == KC // 2 - 1))

        def exp_block(nb, bi, spT):
            blk = P2_BLOCKS[bi]
            if blk[-1] == -2:
                nc.scalar.activation(out=expw[nb][:, 48:49],
                                     in_=spT[:, 48:49], func=AF.Exp,
                                     scale=0.125)
                nc.scalar.activation(out=expw[nb][:, 49:50],
                                     in_=spT[:, 49:50], func=AF.Exp,
                                     scale=0.125, bias=nln16)
            else:
                c0 = blk[0]
                nc.scalar.activation(out=expw[nb][:, c0:c0 + GB],
                                     in_=spT[:, c0:c0 + GB], func=AF.Exp,
                                     scale=0.125)

        def vis_block(nb, bi, cv_tiles, vp):
            """diag build + visAtt accumulate for one block."""
            blk = P2_BLOCKS[bi]
            last = (bi == NBLK - 1)
            dgp = dgpp.tile([128, GB, 128], FP8, tag="dg", name="dg")
            if not last:
                npair = 2
                iv = ident8.unsqueeze(1).to_broadcast([128, GB, 128])
                wv = expw[nb][:, blk[0]:blk[0] + GB].unsqueeze(2).to_broadcast(
                    [128, GB, 128])
                nc.gpsimd.tensor_tensor(out=dgp, in0=iv, in1=wv, op=ALU.mult)
            else:
                # cols 48 (conv) and 49 (fr8); expw col 49 is already exp/16
                npair = 1
                iv = ident8.unsqueeze(1).to_broadcast([128, 2, 128])
                wv = expw[nb][:, 48:50].unsqueeze(2).to_broadcast(
                    [128, 2, 128])
                nc.gpsimd.tensor_tensor(out=dgp[:, 0:2, :], in0=iv, in1=wv,
                                        op=ALU.mult)
            g0 = blk[0] if blk[0] >= 0 else 48
            if g0 < 24:
                cvt, goff = cv_tiles[0], g0
            elif g0 < 48:
                cvt, goff = cv_tiles[1], g0 - 24
            else:
                cvt, goff = cv_tiles[2], 0
            for pj in range(npair):
                for h in range(2):
                    nc.tensor.matmul(
                        vp[:, h * 512:(h + 1) * 512],
                        lhsT=dgp[:, 2 * pj:2 * pj + 2, :],
                        rhs=cvt[:, goff + 2 * pj:goff + 2 * pj + 2,
                                h * 512:(h + 1) * 512],
                        perf_mode=DR,
                        start=(bi == 0 and pj == 0), stop=last,
                        skip_group_check=True)

        def half_tail(nb, vp):
            """softmax denom, vps, sum_T, final dense, out."""
            bsl = slice(nb * 128, (nb + 1) * 128)
            sume = smallp.tile([128, 4], F32, tag="sume", name="sume")
            nc.vector.tensor_reduce(sume[:, 0:1], expw[nb][:, 0:NSC - 1],
                                    axis=mybir.AxisListType.X, op=ALU.add)
            # col 49 is stored as exp/16: add back 16*e49
            nc.gpsimd.tensor_scalar_mul(sume[:, 1:2], expw[nb][:, 49:50],
                                        16.0)
            nc.gpsimd.tensor_add(sume[:, 2:3], sume[:, 0:1], sume[:, 1:2])
            rs = smallp.tile([128, 1], F32, tag="rs", name="rs")
            nc.vector.reciprocal(rs, sume[:, 2:3])
            vps = vpsp.tile([128, R], BF16, tag="vps", name="vps")
            nc.vector.tensor_scalar_mul(vps, vp, rs)
            sum_T = sumtp.tile([128, KC, 128], BF16, tag="sumt", name="sumt")
            for rp in range(2):
                tp = dense_ps.tile([128, 4, 128], BF16, tag="dps", name="tps")
                for i in range(4):
                    rc = rp * 4 + i
                    nc.tensor.transpose(tp[:, i, :],
                                        vps[:, rc * 128:(rc + 1) * 128],
                                        ident)
                for i in range(4):
                    rc = rp * 4 + i
                    nc.vector.tensor_add(sum_T[:, rc, :], tp[:, i, :],
                                         hl_T[:, rc, bsl])
            out_sb = outp.tile([128, R], F32, tag="osb", name="osb")
            for n in range(2):
                yp = dense_ps.tile([128, 2, BS], F32, tag="dps", name="ypf")
                for kc in range(KC):
                    nc.tensor.matmul(yp, lhsT=sum_T[:, kc, :],
                                     rhs=w2ht_box["t"][:, kc,
                                                       n * 512:(n + 1) * 512],
                                     start=(kc == 0), stop=False)
                nc.tensor.matmul(yp, lhsT=ones1,
                                 rhs=b2h_sb[:, n * 512:(n + 1) * 512],
                                 start=False, stop=True)
                nc.scalar.activation(out=out_sb[:, n * 512:(n + 1) * 512],
                                     in_=yp, func=AF.Tanh)
            nc.sync.dma_start(
                out=out_d.rearrange("(nb p) n -> p nb n", p=128)[:, nb, :],
                in_=out_sb)

        # ---------------- schedule ----------------
        # grid blocks stream per half; exp+vis lag by VIS_LAG blocks so the
        # in-order PE/ACT streams never stall on cv DMAs or early scores.
        # cv (conv_feat) dispatches are staged so they never compete with
        # the critical weight/cfes loads at the head.
        pend_tail = None
        for nb in range(2):
            spT = sp_ps.tile([128, 64], F32, tag="spT", name=f"spT{nb}")
            vp = vp_ps.tile([128, R], F32, tag="vp", name=f"vp{nb}")
            cv_tiles = {}
            if nb == 0:
                cv_sched = {2: [0, 2], 5: [1]}
                fill_at, w2_at = 4, 5
                cfes = cfes0
            else:
                cv_sched = {0: [0, 1, 2]}
                fill_at, w2_at = 1, None
                cfes = {0: load_cfe(nb, 0), 1: load_cfe(nb, 1)}
            for bi in range(NBLK):
                for piece in cv_sched.get(bi, []):
                    cv_tiles[piece] = load_cv_piece(nb, piece)
                if bi + 2 < NBLK and bi + 2 not in cfes:
                    cfes[bi + 2] = load_cfe(nb, bi + 2)
                grid_block(nb, bi, cfes.pop(bi), spT)
                if bi == 1 and nb == 0:
                    do_fchain()
                if bi == 1 and pend_tail is not None:
                    half_tail(*pend_tail)
                    pend_tail = None
                if bi == fill_at:
                    fill_fr(nb, cv_tiles)
                if bi == w2_at:
                    w2ht_box["t"] = wpool.tile([128, KC, R], BF16,
                                               name="w2ht", tag="w")
                    nc.sync.dma_start(out=w2ht_box["t"], in_=w2h_r)
                if bi >= VIS_LAG:
                    exp_block(nb, bi - VIS_LAG, spT)
                    vis_block(nb, bi - VIS_LAG, cv_tiles, vp)
            for bi in range(NBLK - VIS_LAG, NBLK):
                exp_block(nb, bi, spT)
                vis_block(nb, bi, cv_tiles, vp)
            pend_tail = (nb, vp)
        half_tail(*pend_tail)

    nc.compile()
    return nc


def _get_nc():
    global _CACHED_NC
    if _CACHED_NC is None:
        _CACHED_NC = _build()
    return _CACHED_NC


def _prep_inputs(h_out, fake_region, conv_feat, conv_feat_embed,
                 Wf1, bf1, Wf2, bf2, Wh1, bh1, Wh2, bh2, Wa, ba, W2h, b2h):
    bf = ml_dtypes.bfloat16
    f8 = ml_dtypes.float8_e4m3
    f32 = np.float32

    wa = np.asarray(Wa, f32).reshape(A)
    sgn = np.where(wa >= 0, 1.0, -1.0).astype(f32)
    awa8 = (8.0 * np.abs(wa)).reshape(KC, 128).T[:, :, None]  # [128, KC, 1]

    def colT(v):
        return np.asarray(v, f32).reshape(KC, 128).T  # [128, KC]

    bp = np.concatenate([
        colT(bh1),
        colT(sgn / 64.0),
        colT(np.asarray(bh2, f32) * sgn),
        colT(16.0 * np.asarray(bf1, f32)),
        colT(sgn / 256.0),
        colT(np.asarray(bf2, f32) * sgn),
    ], axis=1).astype(f32)  # [128, 48]

    xh = np.ascontiguousarray(np.asarray(h_out, f32).T).astype(bf)
    xf8 = np.ascontiguousarray(np.asarray(fake_region, f32).T).astype(f8)
    # cfes: [B, G, A] -> [A, B, G] * sign -> per-core [A, 2, G, 128];
    # chunks c0-3 (a < 512) shipped bf16, c4-7 fp8
    cfe = np.asarray(conv_feat_embed, f32).transpose(2, 0, 1) * sgn[:, None, None]
    cfe = cfe.reshape(A, N_CORES, 2, 128, G).transpose(0, 1, 2, 4, 3)
    cfe = np.ascontiguousarray(cfe)  # [A, cores, 2, G, 128]
    cfesb = cfe[:A // 2].astype(bf)
    cfes8 = cfe[A // 2:].astype(f8)
    cf = np.asarray(conv_feat, f32).astype(f8)

    shared = {
        "wh1t": np.ascontiguousarray(np.asarray(Wh1, f32).T).astype(bf),
        "w2ht": np.ascontiguousarray(np.asarray(W2h, f32).T).astype(bf),
        "wh2t8": np.ascontiguousarray(16.0 * np.asarray(Wh2, f32).T).astype(f8),
        "wf1t8": np.ascontiguousarray(16.0 * np.asarray(Wf1, f32).T).astype(f8),
        "wf2t8": np.ascontiguousarray(16.0 * np.asarray(Wf2, f32).T).astype(f8),
        "bp": bp,
        "awa": awa8.astype(f8),
        "b2h": np.asarray(b2h, f32).reshape(1, R).astype(bf),
    }
    in_maps = []
    for i in range(N_CORES):
        s = slice(i * BS, (i + 1) * BS)
        m = dict(shared)
        m["xh"] = np.ascontiguousarray(xh[:, s])
        m["xf8"] = np.ascontiguousarray(xf8[:, s])
        m["cfesb"] = np.ascontiguousarray(cfesb[:, i])
        m["cfes8"] = np.ascontiguousarray(cfes8[:, i])
        m["cf"] = np.ascontiguousarray(cf[s])
        in_maps.append(m)
    return in_maps


def kernel(**inputs):
    nc = _get_nc()
    in_maps = _prep_inputs(**inputs)
    res = run_bass_kernel_spmd(nc, in_maps, core_ids=list(range(N_CORES)))
    return np.concatenate([res.results[i]["out"] for i in range(N_CORES)],
                          axis=0)


def run_traced(**inputs):
    nc = _get_nc()
    in_maps = _prep_inputs(**inputs)
    res = run_bass_kernel_spmd(nc, in_maps, core_ids=list(range(N_CORES)),
                               trace=True)
    out = np.concatenate([res.results[i]["out"] for i in range(N_CORES)],
                         axis=0)
    return out, res


# revision 29
# speedup vs baseline: 1.1735x; 1.1735x over previous
"""AdaAtt attention kernel for 8 Trainium2 NeuronCores (v3).

Pure data-parallel: batch B=2048 sharded 256 rows/core; weights replicated.
Per core (R=A=1024, G=49):

    fr  = relu(fake_region @ Wf1.T + bf1)
    fre = fr @ Wf2.T + bf2
    hl  = tanh(h_out @ Wh1.T + bh1)
    he  = hl @ Wh2.T + bh2
    scores[g] = Wa . tanh(embed[g] + he)     embed = [conv_feat_embed, fre]
    PI = softmax(scores)
    visAtt = sum_g PI[g] * img[g]            img = [fr, conv_feat]
    out = tanh((visAtt + hl) @ W2h.T + b2h)

v3 vs v2:
  - scores land directly in [b-part, g] PSUM via per-column fp8 DR matmuls
    (stationary = th chunk, moving = 8|wa| column); no DRAM bounce; exp runs
    on [128, 4..50] column tiles.
  - the big cfe+he adds are split DVE (c0-3) / Pool (c4-7); all non-tanh
    PSUM evacuations moved off ACT onto DVE tensor_scalar.
  - Wh2/Wf1/Wf2 + their denses in fp8 DoubleRow (x16 weights, x4
    activations prescale, folded into evac scalars); Wh1/W2h stay bf16.
  - nb-major schedule: half 0's softmax/visAtt/final dense overlap half
    1's tanh grid; vis/exp emission lags the grid by 3 blocks to avoid
    in-order engine stalls; chunked wh1 load starts the PE at ~1.5us.
  - cf streamed as 3 big DMAs per half (128 descriptors of ~25KB each).

Scales: hl8 = 4*hl, xf8 = xf, w*8 = 16*W.T, fr8 = 16*fr (slot 49 of cv;
exp bias -ln(16) on score col 49 compensates; its 1/16 in the softmax
denominator is undone explicitly), he psum = 64x, fre psum = 256x, score
psum = 8x true (awa8 = 8|wa|), exp scale 0.125.
Score/img column layout: 0..48 = conv g, 49 = fre/fr.
"""
import numpy as np
import ml_dtypes
from contextlib import ExitStack

import concourse.bass as bass
import concourse.mybir as mybir
import concourse.tile as tile
from concourse import bacc
from concourse.bass_utils import run_bass_kernel_spmd
from concourse.masks import make_identity

BF16 = mybir.dt.bfloat16
F32 = mybir.dt.float32
FP8 = mybir.dt.float8e4
AF = mybir.ActivationFunctionType
ALU = mybir.AluOpType
DR = mybir.MatmulPerfMode.DoubleRow

N_CORES = 8
B, R, A, G = 2048, 1024, 1024, 49
BS = B // N_CORES          # 256 rows per core
KC = R // 128              # 8 feature chunks
GB = 4                     # g's per block
NSC = G + 1                # score columns (49 conv + fre)
LN16 = float(np.log(16.0))
VIS_LAG = 3                # blocks of lag for exp+vis emission

# 13 blocks per nb-half: 12x4 conv + [48, fre]
P2_BLOCKS = [list(range(s, s + GB)) for s in range(0, 48, GB)] + [[48, -2]]
NBLK = len(P2_BLOCKS)
CV_RANGES = [(0, 24), (24, 48)]        # cvA: pairs 0-11, cvB: pairs 12-23

_CACHED_NC = None


def _build():
    nc = bacc.Bacc("TRN2", target_bir_lowering=False, debug=False,
                   num_devices=N_CORES)

    xh_d = nc.dram_tensor("xh", [R, BS], BF16, kind="ExternalInput").ap()
    xf8_d = nc.dram_tensor("xf8", [R, BS], FP8, kind="ExternalInput").ap()
    cfesb_d = nc.dram_tensor("cfesb", [A // 2, 2, G, 128], BF16,
                             kind="ExternalInput").ap()
    cfes8_d = nc.dram_tensor("cfes8", [A // 2, 2, G, 128], FP8,
                             kind="ExternalInput").ap()
    cf_d = nc.dram_tensor("cf", [BS, G, R], FP8, kind="ExternalInput").ap()
    wh1t_d = nc.dram_tensor("wh1t", [R, R], BF16, kind="ExternalInput").ap()
    w2ht_d = nc.dram_tensor("w2ht", [R, R], BF16, kind="ExternalInput").ap()
    wh2t8_d = nc.dram_tensor("wh2t8", [R, R], FP8, kind="ExternalInput").ap()
    wf1t8_d = nc.dram_tensor("wf1t8", [R, R], FP8, kind="ExternalInput").ap()
    wf2t8_d = nc.dram_tensor("wf2t8", [R, R], FP8, kind="ExternalInput").ap()
    # bias pack [128, 48]: bh1T | (sgn/64)T | (sgn*bh2)T | (16*bf1)T |
    #                      (sgn/256)T | (sgn*bf2)T
    bp_d = nc.dram_tensor("bp", [128, 48], F32, kind="ExternalInput").ap()
    awa_d = nc.dram_tensor("awa", [128, KC, 1], FP8, kind="ExternalInput").ap()
    b2h_d = nc.dram_tensor("b2h", [1, R], BF16, kind="ExternalInput").ap()
    out_d = nc.dram_tensor("out", [BS, R], F32, kind="ExternalOutput").ap()

    cfesb_r = cfesb_d.rearrange("(c p) nb g b -> p c nb g b", p=128)
    cfes8_r = cfes8_d.rearrange("(c p) nb g b -> p c nb g b", p=128)
    wh1_r = wh1t_d.rearrange("(c p) n -> p c n", p=128)
    w2h_r = w2ht_d.rearrange("(c p) n -> p c n", p=128)
    w8_r = {"wh2t8": wh2t8_d.rearrange("(c p) n -> p c n", p=128),
            "wf1t8": wf1t8_d.rearrange("(c p) n -> p c n", p=128),
            "wf2t8": wf2t8_d.rearrange("(c p) n -> p c n", p=128)}

    with ExitStack() as ctx:
        tc = ctx.enter_context(tile.TileContext(nc))
        consts = ctx.enter_context(tc.tile_pool(name="consts", bufs=1))
        wh1p = ctx.enter_context(tc.tile_pool(name="wh1p", bufs=KC))
        wpool = ctx.enter_context(tc.tile_pool(name="wpool", bufs=2))
        acts = ctx.enter_context(tc.tile_pool(name="acts", bufs=1))
        cfebp = ctx.enter_context(tc.tile_pool(name="cfebp", bufs=3))
        cfe8p = ctx.enter_context(tc.tile_pool(name="cfe8p", bufs=3))
        hsp = ctx.enter_context(tc.tile_pool(name="hsp", bufs=2))
        hsfp = ctx.enter_context(tc.tile_pool(name="hsfp", bufs=2))
        thp = ctx.enter_context(tc.tile_pool(name="thp", bufs=4))
        thfp = ctx.enter_context(tc.tile_pool(name="thfp", bufs=2))
        cvp = ctx.enter_context(tc.tile_pool(name="cvp", bufs=2))
        cvcp = ctx.enter_context(tc.tile_pool(name="cvcp", bufs=2))
        dgpp = ctx.enter_context(tc.tile_pool(name="dgpp", bufs=3))
        smallp = ctx.enter_context(tc.tile_pool(name="smallp", bufs=2))
        outp = ctx.enter_context(tc.tile_pool(name="outp", bufs=2))
        sumtp = ctx.enter_context(tc.tile_pool(name="sumtp", bufs=2))
        vpsp = ctx.enter_context(tc.tile_pool(name="vpsp", bufs=2))
        dense_ps = ctx.enter_context(tc.tile_pool(name="dense_ps", bufs=4,
                                                  space="PSUM"))
        sp_ps = ctx.enter_context(tc.tile_pool(name="sp_ps", bufs=2,
                                               space="PSUM"))
        vp_ps = ctx.enter_context(tc.tile_pool(name="vp_ps", bufs=1,
                                               space="PSUM"))

        # ---- constants ----
        ident = consts.tile([128, 128], BF16, name="ident")
        make_identity(nc, ident)
        ident8 = consts.tile([128, 128], FP8, name="ident8")
        make_identity(nc, ident8)
        ones1 = consts.tile([1, 128], BF16, name="ones1")
        nc.vector.memset(ones1, 1.0)
        nln16 = consts.tile([128, 1], F32, name="nln16")
        nc.vector.memset(nln16, -LN16)

        # ---- input loads (SP queue order = priority order) ----
        xh_sb = acts.tile([128, KC, BS], BF16, name="xh_sb")
        nc.sync.dma_start(out=xh_sb,
                          in_=xh_d.rearrange("(c p) b -> p c b", p=128))
        wh1c = []
        for kc in range(KC):
            w = wh1p.tile([128, 1, R], BF16, name=f"wh1c{kc}", tag="wh1c")
            nc.sync.dma_start(out=w, in_=wh1_r[:, kc:kc + 1, :])
            wh1c.append(w)
        bp = consts.tile([128, 48], F32, name="bp")
        nc.sync.dma_start(out=bp, in_=bp_d)
        awa = consts.tile([128, KC, 1], FP8, name="awa")
        nc.sync.dma_start(out=awa, in_=awa_d)
        b2h_sb = consts.tile([1, R], BF16, name="b2h_sb")
        nc.sync.dma_start(out=b2h_sb, in_=b2h_d)
        wh2t8 = wpool.tile([128, KC, R], FP8, name="wh2t8", tag="w")
        nc.sync.dma_start(out=wh2t8, in_=w8_r["wh2t8"])
        xf8_sb = acts.tile([128, KC, BS], FP8, name="xf8_sb")
        nc.sync.dma_start(out=xf8_sb,
                          in_=xf8_d.rearrange("(c p) b -> p c b", p=128))

        # ---- activations ----
        hl_T = acts.tile([128, KC, BS], BF16, name="hl_T")
        hl8 = acts.tile([128, KC, BS], FP8, name="hl8")       # 4*hl
        he_s = acts.tile([128, KC, BS], BF16, name="he_s")    # sgn*(he+bh2)
        fr8 = acts.tile([128, KC, BS], FP8, name="fr8")       # 16*fr
        fr_b = acts.tile([128, KC, BS], BF16, name="fr_b")    # 16*fr (bf16)
        fre_s = acts.tile([128, KC, BS], BF16, name="fre_s")  # sgn*(fre+bf2)
        expw = [acts.tile([128, 64], F32, name=f"expw{nb}") for nb in (0, 1)]

        # PE warm-up: the clock ramps 0.65->1.2->2.4GHz and resets on any
        # >100ns gap; filler matmuls on the identity hold it up through the
        # head's DMA-chasing gaps.
        warm = dense_ps.tile([128, 2, BS], F32, tag="dps", name="warm")

        def pe_warm(n):
            for _ in range(n):
                nc.tensor.matmul(warm[:, 0, 0:128], lhsT=ident, rhs=ident,
                                 start=True, stop=True, skip_group_check=True)

        pe_warm(10)
        # ---- hl dense: bf16; i-sequential groups (one start per bank at a
        # time — PSUM start marks the whole 2KB zero-region pending) ----
        for rp in range(KC // 2):
            ph = dense_ps.tile([128, 2, BS], F32, tag="dps", name=f"hlp{rp}")
            for i in range(2):
                rc = rp * 2 + i
                for kc in range(KC):
                    nc.tensor.matmul(
                        ph[:, i, :],
                        lhsT=wh1c[kc][:, 0, rc * 128:(rc + 1) * 128],
                        rhs=xh_sb[:, kc, :],
                        start=(kc == 0), stop=(kc == KC - 1))
                    if rp == 0 and i == 0:
                        pe_warm(8)   # fill the wh1-chunk arrival gaps
            for i in range(2):
                rc = rp * 2 + i
                nc.scalar.activation(
                    out=hl_T[:, rc, :], in_=ph[:, i, :],
                    func=AF.Tanh, bias=bp[:, rc:rc + 1])

        def dense8(w_sb, x8, evac):
            """fp8 DR dense: psum[rc] = sum_kp w[:,2kp:2kp+2,rc]*x8, evac'd."""
            for rp in range(KC // 2):
                ph = dense_ps.tile([128, 2, BS], F32, tag="dps", name="d8")
                for i in range(2):
                    rc = rp * 2 + i
                    for kp in range(KC // 2):
                        nc.tensor.matmul(
                            ph[:, i, :],
                            lhsT=w_sb[:, 2 * kp:2 * kp + 2,
                                      rc * 128:(rc + 1) * 128],
                            rhs=x8[:, 2 * kp:2 * kp + 2, :],
                            perf_mode=DR,
                            start=(kp == 0), stop=(kp == KC // 2 - 1))
                for i in range(2):
                    evac(rp * 2 + i, ph[:, i, :])

        # earliest cfes blocks: dispatch before the f-chain weight loads so
        # the grid can start as soon as he_s lands
        def load_cfe(nb, bi):
            blk = P2_BLOCKS[bi]
            ng = sum(1 for g in blk if g >= 0)
            cfeb = cfebp.tile([128, 4, GB, 128], BF16, tag="cfb", name="cfb")
            nc.sync.dma_start(out=cfeb[:, :, :ng, :],
                              in_=cfesb_r[:, :, nb, blk[0]:blk[0] + ng, :])
            cfe8 = cfe8p.tile([128, 4, GB, 128], FP8, tag="cf8", name="cf8")
            nc.sync.dma_start(out=cfe8[:, :, :ng, :],
                              in_=cfes8_r[:, :, nb, blk[0]:blk[0] + ng, :])
            return (cfeb, cfe8)

        cfes0 = {bi: load_cfe(0, bi) for bi in range(3)}

        # hl8 = 4*hl as fp8 (DVE)
        nc.vector.tensor_scalar_mul(hl8, hl_T, 4.0)
        pe_warm(30)   # hold the clock through the evac/convert gap
        # he dense (fp8): psum = 64*he ; he_s = (sgn/64)*psum + sgn*bh2
        dense8(wh2t8, hl8, lambda rc, ph: nc.vector.tensor_scalar(
            out=he_s[:, rc, :], in0=ph,
            scalar1=bp[:, 8 + rc:8 + rc + 1],
            scalar2=bp[:, 16 + rc:16 + rc + 1],
            op0=ALU.mult, op1=ALU.add))

        # f-chain is deferred into the early grid so the PE stays hot and
        # the head DMA window carries only wh1/wh2/xh/cfes
        def do_fchain():
            wf1t8 = wpool.tile([128, KC, R], FP8, name="wf1t8", tag="w")
            nc.sync.dma_start(out=wf1t8, in_=w8_r["wf1t8"])
            wf2t8 = wpool.tile([128, KC, R], FP8, name="wf2t8", tag="w")
            nc.sync.dma_start(out=wf2t8, in_=w8_r["wf2t8"])

            # fr dense (fp8): psum = 16*(xf@Wf1.T); fr8 = relu(psum + 16*bf1)
            def fr_evac(rc, ph):
                nc.vector.tensor_scalar(
                    out=fr8[:, rc, :], in0=ph,
                    scalar1=bp[:, 24 + rc:24 + rc + 1], scalar2=0.0,
                    op0=ALU.add, op1=ALU.max)
                nc.vector.tensor_scalar(
                    out=fr_b[:, rc, :], in0=ph,
                    scalar1=bp[:, 24 + rc:24 + rc + 1], scalar2=0.0,
                    op0=ALU.add, op1=ALU.max)
            dense8(wf1t8, xf8_sb, fr_evac)
            # fre dense (fp8): psum = 256*fre; fre_s = (sgn/256)*psum+sgn*bf2
            dense8(wf2t8, fr8, lambda rc, ph: nc.vector.tensor_scalar(
                out=fre_s[:, rc, :], in0=ph,
                scalar1=bp[:, 32 + rc:32 + rc + 1],
                scalar2=bp[:, 40 + rc:40 + rc + 1],
                op0=ALU.mult, op1=ALU.add))

        w2ht_box = {}

        # ---------------- per-half pipeline ----------------
        def load_cv_piece(nb, piece):
            bsl = slice(nb * 128, (nb + 1) * 128)
            if piece < 2:
                ga, gb_ = CV_RANGES[piece]
                cv = cvp.tile([128, gb_ - ga, R], FP8, tag="cv", name="cv")
                nc.gpsimd.dma_start(out=cv, in_=cf_d[bsl, ga:gb_, :])
                return cv
            cvc = cvcp.tile([128, 2, R], FP8, tag="cvc", name="cvc")
            nc.gpsimd.dma_start(out=cvc[:, 0:1, :], in_=cf_d[bsl, 48:49, :])
            return cvc

        def fill_fr(nb, cv_tiles):
            """cv slot 49 <- 64*fr via PE transpose + Pool evac."""
            bsl = slice(nb * 128, (nb + 1) * 128)
            cvc = cv_tiles[2]
            for rp in range(2):
                tp = dense_ps.tile([128, 4, 128], BF16, tag="dps", name="tpf")
                for i in range(4):
                    rc = rp * 4 + i
                    nc.tensor.transpose(tp[:, i, :], fr_b[:, rc, bsl], ident)
                nc.vector.tensor_scalar_mul(
                    cvc[:, 1, rp * 512:(rp + 1) * 512], tp, 1.0)

        def grid_block(nb, bi, cfe, spT):
            """adds + tanh + score columns for one block (no exp here)."""
            blk = P2_BLOCKS[bi]
            bsl = slice(nb * 128, (nb + 1) * 128)
            ng = sum(1 for g in blk if g >= 0)
            gb = len(blk)
            cfeb, cfe8 = cfe
            hs = hsp.tile([128, KC, GB, 128], BF16, tag="hs", name="hs")
            # c0-3: bf16 (DVE 2x); c4-7: fp8, split DVE/Pool by measured rates
            he_lo = he_s[:, 0:4, bsl].unsqueeze(2).to_broadcast(
                [128, 4, ng, 128])
            nc.vector.tensor_add(hs[:, 0:4, :ng, :], cfeb[:, :, :ng, :],
                                 he_lo)
            dc8 = 3 if bi % 2 == 0 else 2
            he_m = he_s[:, 4:4 + dc8, bsl].unsqueeze(2).to_broadcast(
                [128, dc8, ng, 128])
            nc.vector.tensor_add(hs[:, 4:4 + dc8, :ng, :],
                                 cfe8[:, 0:dc8, :ng, :], he_m)
            he_hi = he_s[:, 4 + dc8:8, bsl].unsqueeze(2).to_broadcast(
                [128, 4 - dc8, ng, 128])
            nc.gpsimd.tensor_add(hs[:, 4 + dc8:8, :ng, :],
                                 cfe8[:, dc8:4, :ng, :], he_hi)
            th = thp.tile([128, KC, GB, 128], FP8, tag="th", name="th")
            nc.scalar.activation(out=th[:, :, :ng, :], in_=hs[:, :, :ng, :],
                                 func=AF.Tanh)
            th_f = None
            if gb > ng:  # fre slot
                hsf = hsfp.tile([128, KC, 128], BF16, tag="hsf", name="hsf")
                nc.vector.tensor_add(hsf, fre_s[:, :, bsl], he_s[:, :, bsl])
                th_f = thfp.tile([128, KC, 128], FP8, tag="thf", name="thf")
                nc.scalar.activation(out=th_f, in_=hsf, func=AF.Tanh)
            # score columns: per g, 4 accumulating fp8 DR matmuls
            for gl in range(gb):
                col = NSC - 1 if blk[gl] == -2 else blk[gl]
                for kp in range(KC // 2):
                    lhsT = (th[:, 2 * kp:2 * kp + 2, gl, :] if gl < ng
                            else th_f[:, 2 * kp:2 * kp + 2, :])
                    nc.tensor.matmul(
                        spT[:, col:col + 1], lhsT=lhsT,
                        rhs=awa[:, 2 * kp:2 * kp + 2, :],
                        perf_mode=DR,
                        start=(kp == 0), stop=(kp == KC // 2 - 1))

        def exp_block(nb, bi, spT):
            blk = P2_BLOCKS[bi]
            if blk[-1] == -2:
                nc.scalar.activation(out=expw[nb][:, 48:49],
                                     in_=spT[:, 48:49], func=AF.Exp,
                                     scale=0.125)
                nc.scalar.activation(out=expw[nb][:, 49:50],
                                     in_=spT[:, 49:50], func=AF.Exp,
                                     scale=0.125, bias=nln16)
            else:
                c0 = blk[0]
                nc.scalar.activation(out=expw[nb][:, c0:c0 + GB],
                                     in_=spT[:, c0:c0 + GB], func=AF.Exp,
                                     scale=0.125)

        def vis_block(nb, bi, cv_tiles, vp):
            """diag build + visAtt accumulate for one block."""
            blk = P2_BLOCKS[bi]
            last = (bi == NBLK - 1)
            dgp = dgpp.tile([128, GB, 128], FP8, tag="dg", name="dg")
            if not last:
                npair = 2
                iv = ident8.unsqueeze(1).to_broadcast([128, GB, 128])
                wv = expw[nb][:, blk[0]:blk[0] + GB].unsqueeze(2).to_broadcast(
                    [128, GB, 128])
                nc.gpsimd.tensor_tensor(out=dgp, in0=iv, in1=wv, op=ALU.mult)
            else:
                # cols 48 (conv) and 49 (fr8); expw col 49 is already exp/16
                npair = 1
                iv = ident8.unsqueeze(1).to_broadcast([128, 2, 128])
                wv = expw[nb][:, 48:50].unsqueeze(2).to_broadcast(
                    [128, 2, 128])
                nc.gpsimd.tensor_tensor(out=dgp[:, 0:2, :], in0=iv, in1=wv,
                                        op=ALU.mult)
            g0 = blk[0] if blk[0] >= 0 else 48
            if g0 < 24:
                cvt, goff = cv_tiles[0], g0
            elif g0 < 48:
                cvt, goff = cv_tiles[1], g0 - 24
            else:
                cvt, goff = cv_tiles[2], 0
            for pj in range(npair):
                for h in range(2):
                    nc.tensor.matmul(
                        vp[:, h * 512:(h + 1) * 512],
                        lhsT=dgp[:, 2 * pj:2 * pj + 2, :],
                        rhs=cvt[:, goff + 2 * pj:goff + 2 * pj + 2,
                                h * 512:(h + 1) * 512],
                        perf_mode=DR,
                        start=(bi == 0 and pj == 0), stop=last,
                        skip_group_check=True)

        def half_tail(nb, vp):
            """softmax denom, vps, sum_T, final dense, out."""
            bsl = slice(nb * 128, (nb + 1) * 128)
            sume = smallp.tile([128, 4], F32, tag="sume", name="sume")
            nc.vector.tensor_reduce(sume[:, 0:1], expw[nb][:, 0:NSC - 1],
                                    axis=mybir.AxisListType.X, op=ALU.add)
            # col 49 is stored as exp/16: add back 16*e49
            nc.gpsimd.tensor_scalar_mul(sume[:, 1:2], expw[nb][:, 49:50],
                                        16.0)
            nc.gpsimd.tensor_add(sume[:, 2:3], sume[:, 0:1], sume[:, 1:2])
            rs = smallp.tile([128, 1], F32, tag="rs", name="rs")
            nc.vector.reciprocal(rs, sume[:, 2:3])
            vps = vpsp.tile([128, R], BF16, tag="vps", name="vps")
            nc.vector.tensor_scalar_mul(vps, vp, rs)
            sum_T = sumtp.tile([128, KC, 128], BF16, tag="sumt", name="sumt")
            for rp in range(2):
                tp = dense_ps.tile([128, 4, 128], BF16, tag="dps", name="tps")
                for i in range(4):
                    rc = rp * 4 + i
                    nc.tensor.transpose(tp[:, i, :],
                                        vps[:, rc * 128:(rc + 1) * 128],
                                        ident)
                for i in range(4):
                    rc = rp * 4 + i
                    nc.vector.tensor_add(sum_T[:, rc, :], tp[:, i, :],
                                         hl_T[:, rc, bsl])
            out_sb = outp.tile([128, R], F32, tag="osb", name="osb")
            for n in range(2):
                yp = dense_ps.tile([128, 2, BS], F32, tag="dps", name="ypf")
                for kc in range(KC):
                    nc.tensor.matmul(yp, lhsT=sum_T[:, kc, :],
                                     rhs=w2ht_box["t"][:, kc,
                                                       n * 512:(n + 1) * 512],
                                     start=(kc == 0), stop=False)
                nc.tensor.matmul(yp, lhsT=ones1,
                                 rhs=b2h_sb[:, n * 512:(n + 1) * 512],
                                 start=False, stop=True)
                nc.scalar.activation(out=out_sb[:, n * 512:(n + 1) * 512],
                                     in_=yp, func=AF.Tanh)
            nc.sync.dma_start(
                out=out_d.rearrange("(nb p) n -> p nb n", p=128)[:, nb, :],
                in_=out_sb)

        # ---------------- schedule ----------------
        # grid blocks stream per half; exp+vis lag by VIS_LAG blocks so the
        # in-order PE/ACT streams never stall on cv DMAs or early scores.
        # cv (conv_feat) dispatches are staged so they never compete with
        # the critical weight/cfes loads at the head.
        pend_tail = None
        for nb in range(2):
            spT = sp_ps.tile([128, 64], F32, tag="spT", name=f"spT{nb}")
            vp = vp_ps.tile([128, R], F32, tag="vp", name=f"vp{nb}")
            cv_tiles = {}
            if nb == 0:
                cv_sched = {2: [0, 2], 5: [1]}
                fill_at, w2_at = 4, 5
                cfes = cfes0
            else:
                cv_sched = {0: [0, 1, 2]}
                fill_at, w2_at = 1, None
                cfes = {0: load_cfe(nb, 0), 1: load_cfe(nb, 1)}
            for bi in range(NBLK):
                for piece in cv_sched.get(bi, []):
                    cv_tiles[piece] = load_cv_piece(nb, piece)
                if bi + 2 < NBLK and bi + 2 not in cfes:
                    cfes[bi + 2] = load_cfe(nb, bi + 2)
                grid_block(nb, bi, cfes.pop(bi), spT)
                if bi == 1 and nb == 0:
                    do_fchain()
                if bi == 1 and pend_tail is not None:
                    half_tail(*pend_tail)
                    pend_tail = None
                if bi == fill_at:
                    fill_fr(nb, cv_tiles)
                if bi == w2_at:
                    w2ht_box["t"] = wpool.tile([128, KC, R], BF16,
                                               name="w2ht", tag="w")
                    nc.sync.dma_start(out=w2ht_box["t"], in_=w2h_r)
                if bi >= VIS_LAG:
                    exp_block(nb, bi - VIS_LAG, spT)
                    vis_block(nb, bi - VIS_LAG, cv_tiles, vp)
            for bi in range(NBLK - VIS_LAG, NBLK):
                exp_block(nb, bi, spT)
                vis_block(nb, bi, cv_tiles, vp)
            pend_tail = (nb, vp)
        half_tail(*pend_tail)

    nc.compile()
    return nc


def _get_nc():
    global _CACHED_NC
    if _CACHED_NC is None:
        _CACHED_NC = _build()
    return _CACHED_NC


def _prep_inputs(h_out, fake_region, conv_feat, conv_feat_embed,
                 Wf1, bf1, Wf2, bf2, Wh1, bh1, Wh2, bh2, Wa, ba, W2h, b2h):
    bf = ml_dtypes.bfloat16
    f8 = ml_dtypes.float8_e4m3
    f32 = np.float32

    wa = np.asarray(Wa, f32).reshape(A)
    sgn = np.where(wa >= 0, 1.0, -1.0).astype(f32)
    awa8 = (8.0 * np.abs(wa)).reshape(KC, 128).T[:, :, None]  # [128, KC, 1]

    def colT(v):
        return np.asarray(v, f32).reshape(KC, 128).T  # [128, KC]

    bp = np.concatenate([
        colT(bh1),
        colT(sgn / 64.0),
        colT(np.asarray(bh2, f32) * sgn),
        colT(16.0 * np.asarray(bf1, f32)),
        colT(sgn / 256.0),
        colT(np.asarray(bf2, f32) * sgn),
    ], axis=1).astype(f32)  # [128, 48]

    xh = np.ascontiguousarray(np.asarray(h_out, f32).T).astype(bf)
    xf8 = np.ascontiguousarray(np.asarray(fake_region, f32).T).astype(f8)
    # cfes: [B, G, A] -> [A, B, G] * sign -> per-core [A, 2, G, 128];
    # chunks c0-3 (a < 512) shipped bf16, c4-7 fp8
    cfe = np.asarray(conv_feat_embed, f32).transpose(2, 0, 1) * sgn[:, None, None]
    cfe = cfe.reshape(A, N_CORES, 2, 128, G).transpose(0, 1, 2, 4, 3)
    cfe = np.ascontiguousarray(cfe)  # [A, cores, 2, G, 128]
    cfesb = cfe[:A // 2].astype(bf)
    cfes8 = cfe[A // 2:].astype(f8)
    cf = np.asarray(conv_feat, f32).astype(f8)

    shared = {
        "wh1t": np.ascontiguousarray(np.asarray(Wh1, f32).T).astype(bf),
        "w2ht": np.ascontiguousarray(np.asarray(W2h, f32).T).astype(bf),
        "wh2t8": np.ascontiguousarray(16.0 * np.asarray(Wh2, f32).T).astype(f8),
        "wf1t8": np.ascontiguousarray(16.0 * np.asarray(Wf1, f32).T).astype(f8),
        "wf2t8": np.ascontiguousarray(16.0 * np.asarray(Wf2, f32).T).astype(f8),
        "bp": bp,
        "awa": awa8.astype(f8),
        "b2h": np.asarray(b2h, f32).reshape(1, R).astype(bf),
    }
    in_maps = []
    for i in range(N_CORES):
        s = slice(i * BS, (i + 1) * BS)
        m = dict(shared)
        m["xh"] = np.ascontiguousarray(xh[:, s])
        m["xf8"] = np.ascontiguousarray(xf8[:, s])
        m["cfesb"] = np.ascontiguousarray(cfesb[:, i])
        m["cfes8"] = np.ascontiguousarray(cfes8[:, i])
        m["cf"] = np.ascontiguousarray(cf[s])
        in_maps.append(m)
    return in_maps


def kernel(**inputs):
    nc = _get_nc()
    in_maps = _prep_inputs(**inputs)
    res = run_bass_kernel_spmd(nc, in_maps, core_ids=list(range(N_CORES)))
    return np.concatenate([res.results[i]["out"] for i in range(N_CORES)],
                          axis=0)


def run_traced(**inputs):
    nc = _get_nc()
    in_maps = _prep_inputs(**inputs)
    res = run_bass_kernel_spmd(nc, in_maps, core_ids=list(range(N_CORES)),
                               trace=True)
    out = np.concatenate([res.results[i]["out"] for i in range(N_CORES)],
                         axis=0)
    return out, res
